# revision 10
# baseline (speedup 1.0000x reference)
"""CenterLoss Trainium2 kernel.

Reference computes, for x[B,D], labels[B], centers[C,D]:
    distmat[b,c] = ||x_b||^2 + ||c_c||^2 - 2<x_b, c_c>
    dist = where(labels[b]==c, distmat, 0)
    loss = clip(dist, 1e-12, 1e12).sum() / B

Only one entry per row survives the mask: d_b = ||x_b - centers[labels_b]||^2.
The other C-1 zeros per row are clamped to 1e-12, contributing the constant
B*(C-1)*1e-12 to the sum.  So:

    loss = ( sum_b clip(d_b, 1e-12, 1e12) ) / B  +  (C-1)*1e-12

No [B,C] distmat needed: gather centers[labels] (indirect DMA), squared
distance per row, clip, reduce.  Data-parallel over batch across 8 cores;
centers stay in HBM and only the labeled rows are read (indirect gather).

Per-core layout (fixed by dma_gather): row r of the 1024-row shard lives at
partition p = r%128, free slot j = r//128; idxs are int16 wrapped as
[16, 64] (idx i at partition i%16, slot i//16) replicated across the 8
GPSIMD-core partition groups -> [128, 64].
"""

import numpy as np

B, C, D = 8192, 10000, 128
N_CORES = 8
RPC = B // N_CORES  # rows per core
P = 128
J = RPC // P  # free slots per partition

CLIP_LO = 1e-12
CLIP_HI = 1e12
MASK_CONST = (C - 1) * CLIP_LO  # clamped masked-out zeros, after /B

_cache = {}


def _build(dbg=False):
    import concourse.bacc as bacc
    import concourse.bass as bass
    import concourse.mybir as mybir
    import concourse.tile as tile

    f32 = mybir.dt.float32
    i32 = mybir.dt.int32

    nc = bacc.Bacc("TRN2", target_bir_lowering=False, debug=False)

    i16 = mybir.dt.int16
    x_d = nc.dram_tensor("x", [RPC, D], f32, kind="ExternalInput")
    lab_d = nc.dram_tensor("labels", [P, RPC // 16], i16, kind="ExternalInput")
    cen_d = nc.dram_tensor("centers", [C, D], f32, kind="ExternalInput")
    out_d = nc.dram_tensor("out", [1, 1], f32, kind="ExternalOutput")
    if dbg:
        dbg_ct = nc.dram_tensor("dbg_ct", [P, J, D], f32, kind="ExternalOutput")
        dbg_xt = nc.dram_tensor("dbg_xt", [P, J, D], f32, kind="ExternalOutput")
        dbg_dsum = nc.dram_tensor("dbg_dsum", [P, J], f32, kind="ExternalOutput")
        dbg_dtot = nc.dram_tensor("dbg_dtot", [P, 1], f32, kind="ExternalOutput")

    N_CHUNK = 2  # split gather/compute for DMA/compute overlap
    JC = J // N_CHUNK

    with tile.TileContext(nc) as tc:
        with (
            tc.tile_pool(name="sbuf", bufs=1) as pool,
            tc.tile_pool(name="psum", bufs=1, space="PSUM") as psum_pool,
        ):
            xt = pool.tile([P, J, D], f32)
            ct = pool.tile([P, J, D], f32)
            diff = pool.tile([P, J, D], f32)
            sq = pool.tile([P, J, D], f32)
            it = pool.tile([P, RPC // 16], i16)
            dsum = pool.tile([P, J], f32)
            dclip = pool.tile([P, J], f32)
            dtot = pool.tile([P, 1], f32)
            ones = pool.tile([P, 1], f32)
            res = pool.tile([1, 1], f32)

            nc.sync.dma_start(out=it[:], in_=lab_d[:, :])
            nc.vector.memset(ones[:], 1.0)

            # x rows land to match the gather layout: row r -> (p=r%128, j=r//128)
            x_ap = x_d[:, :].rearrange("(j p) d -> p j d", p=P)
            nc.sync.dma_start(out=xt[:, :, :], in_=x_ap[:, :, :])
            nc.gpsimd.dma_gather(
                out_ap=ct[:],
                in_ap=cen_d[:, :],
                idxs_ap=it[:],
                num_idxs=RPC,
                num_idxs_reg=RPC,
                elem_size=D,
            )
            for h in range(N_CHUNK):
                js = slice(h * JC, (h + 1) * JC)
                nc.vector.tensor_tensor(
                    out=diff[:, js, :],
                    in0=xt[:, js, :],
                    in1=ct[:, js, :],
                    op=mybir.AluOpType.subtract,
                )
                for j in range(h * JC, (h + 1) * JC):
                    nc.scalar.activation(
                        out=sq[:, j, :],
                        in_=diff[:, j, :],
                        func=mybir.ActivationFunctionType.Square,
                        accum_out=dsum[:, j : j + 1],
                    )

            # clip each per-row distance, then sum the J slots per partition
            nc.vector.tensor_scalar(
                out=dclip[:],
                in0=dsum[:],
                scalar1=CLIP_LO,
                scalar2=CLIP_HI,
                op0=mybir.AluOpType.max,
                op1=mybir.AluOpType.min,
            )
            nc.vector.tensor_reduce(
                out=dtot[:],
                in_=dclip[:],
                axis=mybir.AxisListType.X,
                op=mybir.AluOpType.add,
            )
            # cross-partition sum via PE: [1,1] = dtot[128,1].T @ ones[128,1]
            pt = psum_pool.tile([1, 1], f32)
            nc.tensor.matmul(out=pt[:], lhsT=dtot[:], rhs=ones[:], start=True, stop=True)
            nc.scalar.activation(
                out=res[:],
                in_=pt[:],
                func=mybir.ActivationFunctionType.Copy,
                scale=1.0 / B,
            )
            nc.sync.dma_start(out=out_d[:, :], in_=res[:])
            if dbg:
                nc.sync.dma_start(out=dbg_ct[:, :, :], in_=ct[:])
                nc.sync.dma_start(out=dbg_xt[:, :, :], in_=xt[:])
                nc.sync.dma_start(out=dbg_dsum[:, :], in_=dsum[:])
                nc.sync.dma_start(out=dbg_dtot[:, :], in_=dtot[:])

    nc.compile()
    return nc


def _get_nc():
    if "nc" not in _cache:
        _cache["nc"] = _build()
    return _cache["nc"]


def _make_in_maps(x, labels, centers):
    x = np.ascontiguousarray(np.asarray(x, dtype=np.float32))
    labels = np.asarray(labels).astype(np.int16)
    centers = np.ascontiguousarray(np.asarray(centers, dtype=np.float32))
    in_maps = []
    for i in range(N_CORES):
        sl = slice(i * RPC, (i + 1) * RPC)
        # dma_gather idx wrap: idx i at partition i%16, slot i//16,
        # replicated across the 8 GPSIMD-core partition groups
        wrapped = np.ascontiguousarray(labels[sl].reshape(RPC // 16, 16).T)
        in_maps.append(
            {
                "x": x[sl],
                "labels": np.ascontiguousarray(np.tile(wrapped, (8, 1))),
                "centers": centers,
            }
        )
    return in_maps


def _run(in_maps, trace=False, **kwargs):
    from concourse.bass_utils import run_bass_kernel_spmd

    nc = _get_nc()
    return run_bass_kernel_spmd(
        nc, in_maps, core_ids=list(range(N_CORES)), trace=trace, **kwargs
    )


def kernel(x, labels, centers):
    res = _run(_make_in_maps(x, labels, centers))
    total = np.float32(0.0)
    for r in res.results:
        total += np.float32(r["out"].reshape(()))
    return np.asarray(total + np.float32(MASK_CONST), dtype=np.float32)


# revision 23
# speedup vs baseline: 1.4391x; 1.4391x over previous
"""CenterLoss Trainium2 kernel.

Reference computes, for x[B,D], labels[B], centers[C,D]:
    distmat[b,c] = ||x_b||^2 + ||c_c||^2 - 2<x_b, c_c>
    dist = where(labels[b]==c, distmat, 0)
    loss = clip(dist, 1e-12, 1e12).sum() / B

Only one entry per row survives the mask: d_b = ||x_b - centers[labels_b]||^2.
The other C-1 zeros per row are clamped to 1e-12, contributing the constant
B*(C-1)*1e-12 to the sum.  So:

    loss = ( sum_b clip(d_b, 1e-12, 1e12) ) / B  +  (C-1)*1e-12

No [B,C] distmat needed: gather centers[labels] (indirect DMA), squared
distance per row, clip, reduce.  Data-parallel over batch across 8 cores;
centers stay in HBM and only the labeled rows are read (indirect gather).
Raw bacc (no Tile) with manual semaphores to minimize preamble/epilogue.

Per-core layout: row r of the 1024-row shard lives at partition p = r//8,
free slot j = r%8 (x and label loads are contiguous DMAs; gather j fetches
rows {p*8+j} via per-partition offsets it[:, j]).
"""

import numpy as np

B, C, D = 8192, 10000, 128
N_CORES = 8
RPC = B // N_CORES  # rows per core
P = 128
J = RPC // P  # free slots per partition

CLIP_LO = 1e-12
CLIP_HI = 1e12
MASK_CONST = (C - 1) * CLIP_LO  # clamped masked-out zeros, after /B

_cache = {}


def _build(fake_gather=False):
    from contextlib import ExitStack

    import concourse.bacc as bacc
    import concourse.bass as bass
    import concourse.mybir as mybir

    f32 = mybir.dt.float32
    i32 = mybir.dt.int32

    nc = bacc.Bacc("TRN2", target_bir_lowering=False, debug=False)

    x_d = nc.dram_tensor("x", [RPC, D], f32, kind="ExternalInput")
    lab_d = nc.dram_tensor("labels", [P, J], i32, kind="ExternalInput")
    cen_d = nc.dram_tensor("centers", [C, D], f32, kind="ExternalInput")
    out_d = nc.dram_tensor("out", [1, 1], f32, kind="ExternalOutput")

    with (
        ExitStack() as ctx,
        nc.sbuf_tensor("xt", [P, J, D], f32) as xt,
        nc.sbuf_tensor("ct", [P, J, D], f32) as ct,
        nc.sbuf_tensor("sq", [P, J, D], f32) as sq,
        nc.sbuf_tensor("it", [P, J], i32) as it,
        nc.sbuf_tensor("dsum", [P, J], f32) as dsum,
        nc.sbuf_tensor("dclip", [P, J], f32) as dclip,
        nc.sbuf_tensor("dtot", [P, 1], f32) as dtot,
        nc.sbuf_tensor("onesb", [P, 1], f32) as onesb,
        nc.sbuf_tensor("res", [1, 1], f32) as res,
        nc.psum_tensor("pacc", [1, 1], f32) as pacc,
        nc.semaphore("s_idx") as s_idx,
        nc.semaphore("s_x") as s_x,
        nc.semaphore("s_v") as s_v,
        nc.semaphore("s_mm") as s_mm,
        nc.semaphore("s_res") as s_res,
        nc.semaphore("s_out") as s_out,
        nc.Block() as block,
    ):
        s_g = [ctx.enter_context(nc.semaphore(f"s_g{j}")) for j in range(J)]  # noqa: ANT232

        @block.sync
        def _(sync):
            sync.dma_start(out=it[:], in_=lab_d[:, :]).then_inc(s_idx, 16)
            x_ap = x_d[:, :].rearrange("(p j) d -> p j d", p=P)
            sync.dma_start(out=xt[:], in_=x_ap).then_inc(s_x, 16)
            sync.wait_ge(s_res, 1)
            sync.dma_start(out=out_d[:, :], in_=res[:]).then_inc(s_out, 16)
            sync.wait_ge(s_out, 16)

        @block.gpsimd
        def _(gpsimd):
            gpsimd.wait_ge(s_idx, 16)
            for j in range(J):
                if fake_gather:
                    gpsimd.dma_start(
                        out=ct[:, j, :], in_=cen_d[j * P : (j + 1) * P, :]
                    ).then_inc(s_g[j], 16)
                else:
                    gpsimd.indirect_dma_start(
                        out=ct[:, j, :],
                        out_offset=None,
                        in_=cen_d[:, :],
                        in_offset=bass.IndirectOffsetOnAxis(ap=it[:, j : j + 1], axis=0),
                    ).then_inc(s_g[j], 16)

        @block.vector
        def _(vector):
            vector.memset(onesb[:], 1.0 / B)
            vector.wait_ge(s_x, 16)
            for j in range(J):
                vector.wait_ge(s_g[j], 16)
                vector.tensor_tensor(
                    out=sq[:, j, :],
                    in0=xt[:, j, :],
                    in1=ct[:, j, :],
                    op=mybir.AluOpType.subtract,
                )
                vector.drain()  # DVE pipeline: sq_j write -> read below
                vector.tensor_tensor(
                    out=sq[:, j, :],
                    in0=sq[:, j, :],
                    in1=sq[:, j, :],
                    op=mybir.AluOpType.mult,
                )
                vector.drain()
                vector.tensor_reduce(
                    out=dsum[:, j : j + 1],
                    in_=sq[:, j, :],
                    axis=mybir.AxisListType.X,
                    op=mybir.AluOpType.add,
                )
            vector.drain()
            vector.tensor_scalar(
                out=dclip[:],
                in0=dsum[:],
                scalar1=CLIP_LO,
                scalar2=CLIP_HI,
                op0=mybir.AluOpType.max,
                op1=mybir.AluOpType.min,
            )
            vector.drain()
            vector.tensor_reduce(
                out=dtot[:],
                in_=dclip[:],
                axis=mybir.AxisListType.X,
                op=mybir.AluOpType.add,
            ).then_inc(s_v, 1)
            # after PE sums partitions into PSUM, move to SBUF for the out-DMA
            vector.wait_ge(s_mm, 1)
            vector.tensor_copy(out=res[:], in_=pacc[:]).then_inc(s_res, 1)

        @block.tensor
        def _(tensor):
            tensor.wait_ge(s_v, 1)
            nc.tensor.matmul(
                out=pacc[:], lhsT=dtot[:], rhs=onesb[:], start=True, stop=True
            ).then_inc(s_mm, 1)

    nc.compile()
    return nc


def _get_nc():
    if "nc" not in _cache:
        _cache["nc"] = _build()
    return _cache["nc"]


def _make_in_maps(x, labels, centers):
    x = np.ascontiguousarray(np.asarray(x, dtype=np.float32))
    labels = np.asarray(labels).astype(np.int32)
    centers = np.ascontiguousarray(np.asarray(centers, dtype=np.float32))
    in_maps = []
    for i in range(N_CORES):
        sl = slice(i * RPC, (i + 1) * RPC)
        in_maps.append(
            {
                "x": x[sl],
                "labels": np.ascontiguousarray(labels[sl].reshape(P, J)),
                "centers": centers,
            }
        )
    return in_maps


def _run(in_maps, trace=False, **kwargs):
    from concourse.bass_utils import run_bass_kernel_spmd

    nc = _get_nc()
    return run_bass_kernel_spmd(
        nc, in_maps, core_ids=list(range(N_CORES)), trace=trace, **kwargs
    )


def kernel(x, labels, centers):
    res = _run(_make_in_maps(x, labels, centers))
    total = np.float32(0.0)
    for r in res.results:
        total += np.float32(r["out"].reshape(()))
    return np.asarray(total + np.float32(MASK_CONST), dtype=np.float32)


# revision 24
# speedup vs baseline: 1.4595x; 1.0142x over previous
"""CenterLoss Trainium2 kernel.

Reference computes, for x[B,D], labels[B], centers[C,D]:
    distmat[b,c] = ||x_b||^2 + ||c_c||^2 - 2<x_b, c_c>
    dist = where(labels[b]==c, distmat, 0)
    loss = clip(dist, 1e-12, 1e12).sum() / B

Only one entry per row survives the mask: d_b = ||x_b - centers[labels_b]||^2.
The other C-1 zeros per row are clamped to 1e-12, contributing the constant
B*(C-1)*1e-12 to the sum.  So:

    loss = ( sum_b clip(d_b, 1e-12, 1e12) ) / B  +  (C-1)*1e-12

No [B,C] distmat needed: gather centers[labels] (indirect DMA), squared
distance per row, clip, reduce.  Data-parallel over batch across 8 cores;
centers stay in HBM and only the labeled rows are read (indirect gather).
Raw bacc (no Tile) with manual semaphores to minimize preamble/epilogue.

Per-core layout: row r of the 1024-row shard lives at partition p = r//8,
free slot j = r%8 (x and label loads are contiguous DMAs; gather j fetches
rows {p*8+j} via per-partition offsets it[:, j]).
"""

import numpy as np

B, C, D = 8192, 10000, 128
N_CORES = 8
RPC = B // N_CORES  # rows per core
P = 128
J = RPC // P  # free slots per partition

CLIP_LO = 1e-12
CLIP_HI = 1e12
MASK_CONST = (C - 1) * CLIP_LO  # clamped masked-out zeros, after /B

_cache = {}


def _build(fake_gather=False, skip_init_barrier=True, skip_exit_barrier=False):
    from contextlib import ExitStack

    import concourse.bacc as bacc
    import concourse.bass as bass
    import concourse.mybir as mybir

    f32 = mybir.dt.float32
    i32 = mybir.dt.int32

    class _FastBacc(bacc.Bacc):
        # engines don't touch each other's state before the first real
        # cross-engine semaphore, so the init-time (const-ap memset) and
        # optionally the Block-exit all-engine barriers can be elided
        _skip_barrier = True

        def all_engine_barrier(self, **kw):
            if self._skip_barrier:
                return
            return super().all_engine_barrier(**kw)

    nc = _FastBacc("TRN2", target_bir_lowering=False, debug=False)
    if not skip_init_barrier:
        bacc.Bacc.all_engine_barrier(nc)
    if not skip_exit_barrier:
        nc._skip_barrier = False

    x_d = nc.dram_tensor("x", [RPC, D], f32, kind="ExternalInput")
    lab_d = nc.dram_tensor("labels", [P, J], i32, kind="ExternalInput")
    cen_d = nc.dram_tensor("centers", [C, D], f32, kind="ExternalInput")
    out_d = nc.dram_tensor("out", [1, 1], f32, kind="ExternalOutput")

    with (
        ExitStack() as ctx,
        nc.sbuf_tensor("xt", [P, J, D], f32) as xt,
        nc.sbuf_tensor("ct", [P, J, D], f32) as ct,
        nc.sbuf_tensor("sq", [P, J, D], f32) as sq,
        nc.sbuf_tensor("it", [P, J], i32) as it,
        nc.sbuf_tensor("dsum", [P, J], f32) as dsum,
        nc.sbuf_tensor("dclip", [P, J], f32) as dclip,
        nc.sbuf_tensor("dtot", [P, 1], f32) as dtot,
        nc.sbuf_tensor("onesb", [P, 1], f32) as onesb,
        nc.sbuf_tensor("res", [1, 1], f32) as res,
        nc.psum_tensor("pacc", [1, 1], f32) as pacc,
        nc.semaphore("s_idx") as s_idx,
        nc.semaphore("s_x") as s_x,
        nc.semaphore("s_v") as s_v,
        nc.semaphore("s_mm") as s_mm,
        nc.semaphore("s_res") as s_res,
        nc.semaphore("s_out") as s_out,
        nc.Block() as block,
    ):
        s_g = [ctx.enter_context(nc.semaphore(f"s_g{j}")) for j in range(J)]  # noqa: ANT232

        @block.sync
        def _(sync):
            sync.dma_start(out=it[:], in_=lab_d[:, :]).then_inc(s_idx, 16)
            x_ap = x_d[:, :].rearrange("(p j) d -> p j d", p=P)
            sync.dma_start(out=xt[:], in_=x_ap).then_inc(s_x, 16)
            sync.wait_ge(s_res, 1)
            sync.dma_start(out=out_d[:, :], in_=res[:]).then_inc(s_out, 16)
            sync.wait_ge(s_out, 16)

        @block.gpsimd
        def _(gpsimd):
            gpsimd.wait_ge(s_idx, 16)
            for j in range(J):
                if fake_gather:
                    gpsimd.dma_start(
                        out=ct[:, j, :], in_=cen_d[j * P : (j + 1) * P, :]
                    ).then_inc(s_g[j], 16)
                else:
                    gpsimd.indirect_dma_start(
                        out=ct[:, j, :],
                        out_offset=None,
                        in_=cen_d[:, :],
                        in_offset=bass.IndirectOffsetOnAxis(ap=it[:, j : j + 1], axis=0),
                    ).then_inc(s_g[j], 16)

        @block.vector
        def _(vector):
            vector.memset(onesb[:], 1.0 / B)
            vector.wait_ge(s_x, 16)
            for j in range(J):
                vector.wait_ge(s_g[j], 16)
                vector.tensor_tensor(
                    out=sq[:, j, :],
                    in0=xt[:, j, :],
                    in1=ct[:, j, :],
                    op=mybir.AluOpType.subtract,
                )
                vector.drain()  # DVE pipeline: sq_j write -> read below
                vector.tensor_tensor(
                    out=sq[:, j, :],
                    in0=sq[:, j, :],
                    in1=sq[:, j, :],
                    op=mybir.AluOpType.mult,
                )
                vector.drain()
                vector.tensor_reduce(
                    out=dsum[:, j : j + 1],
                    in_=sq[:, j, :],
                    axis=mybir.AxisListType.X,
                    op=mybir.AluOpType.add,
                )
            vector.drain()
            vector.tensor_scalar(
                out=dclip[:],
                in0=dsum[:],
                scalar1=CLIP_LO,
                scalar2=CLIP_HI,
                op0=mybir.AluOpType.max,
                op1=mybir.AluOpType.min,
            )
            vector.drain()
            vector.tensor_reduce(
                out=dtot[:],
                in_=dclip[:],
                axis=mybir.AxisListType.X,
                op=mybir.AluOpType.add,
            ).then_inc(s_v, 1)
            # after PE sums partitions into PSUM, move to SBUF for the out-DMA
            vector.wait_ge(s_mm, 1)
            vector.tensor_copy(out=res[:], in_=pacc[:]).then_inc(s_res, 1)

        @block.tensor
        def _(tensor):
            tensor.wait_ge(s_v, 1)
            nc.tensor.matmul(
                out=pacc[:], lhsT=dtot[:], rhs=onesb[:], start=True, stop=True
            ).then_inc(s_mm, 1)

    nc.compile()
    return nc


def _get_nc():
    if "nc" not in _cache:
        _cache["nc"] = _build()
    return _cache["nc"]


def _make_in_maps(x, labels, centers):
    x = np.ascontiguousarray(np.asarray(x, dtype=np.float32))
    labels = np.asarray(labels).astype(np.int32)
    centers = np.ascontiguousarray(np.asarray(centers, dtype=np.float32))
    in_maps = []
    for i in range(N_CORES):
        sl = slice(i * RPC, (i + 1) * RPC)
        in_maps.append(
            {
                "x": x[sl],
                "labels": np.ascontiguousarray(labels[sl].reshape(P, J)),
                "centers": centers,
            }
        )
    return in_maps


def _run(in_maps, trace=False, **kwargs):
    from concourse.bass_utils import run_bass_kernel_spmd

    nc = _get_nc()
    return run_bass_kernel_spmd(
        nc, in_maps, core_ids=list(range(N_CORES)), trace=trace, **kwargs
    )


def kernel(x, labels, centers):
    res = _run(_make_in_maps(x, labels, centers))
    total = np.float32(0.0)
    for r in res.results:
        total += np.float32(r["out"].reshape(()))
    return np.asarray(total + np.float32(MASK_CONST), dtype=np.float32)


# revision 25
# speedup vs baseline: 1.5381x; 1.0539x over previous
"""CenterLoss Trainium2 kernel.

Reference computes, for x[B,D], labels[B], centers[C,D]:
    distmat[b,c] = ||x_b||^2 + ||c_c||^2 - 2<x_b, c_c>
    dist = where(labels[b]==c, distmat, 0)
    loss = clip(dist, 1e-12, 1e12).sum() / B

Only one entry per row survives the mask: d_b = ||x_b - centers[labels_b]||^2.
The other C-1 zeros per row are clamped to 1e-12, contributing the constant
B*(C-1)*1e-12 to the sum.  So:

    loss = ( sum_b clip(d_b, 1e-12, 1e12) ) / B  +  (C-1)*1e-12

No [B,C] distmat needed: gather centers[labels] (indirect DMA), squared
distance per row (scaled by 1/B, with the clip bounds scaled to match),
clip, reduce.  Data-parallel over batch across 8 cores; centers stay in
HBM and only the labeled rows are read (indirect gather).

Raw bacc, no Tile, no Block: engine programs are emitted straight into the
main basic block (single IRAM block, no body ifetch, no exit barrier) with
manual semaphores.  Only Sync (input/output DMA), GpSimd (gather + final
cross-partition reduce) and Vector are used; the Tensor engine is unused so
its preamble (a config write plus a ~2.4us settle that gates the entry
barrier) is skipped.

Per-core layout: row r of the 1024-row shard lives at partition p = r//8,
free slot j = r%8 (x and label loads are contiguous DMAs; gather j fetches
rows {p*8+j} via per-partition offsets it[:, j]).
"""

import numpy as np

B, C, D = 8192, 10000, 128
N_CORES = 8
RPC = B // N_CORES  # rows per core
P = 128
J = RPC // P  # free slots per partition

CLIP_LO = 1e-12
CLIP_HI = 1e12
MASK_CONST = (C - 1) * CLIP_LO  # clamped masked-out zeros, after /B

_cache = {}


def _build():
    from contextlib import ExitStack

    import concourse.bacc as bacc
    import concourse.bass as bass
    import concourse.mybir as mybir

    f32 = mybir.dt.float32
    i32 = mybir.dt.int32

    class _FastBacc(bacc.Bacc):
        # the init-time all-engine barrier only guards the const-ap
        # memsets, which this kernel never reads — skip it
        def all_engine_barrier(self, **kw):
            return

    # PE is unused; its preamble's config-write + settle would gate the
    # runtime entry barrier for ~2.8us
    pe_preamble = bass.BassTensorEngine.preamble
    bass.BassTensorEngine.preamble = lambda self: None
    try:
        nc = _FastBacc("TRN2", target_bir_lowering=False, debug=False)
    finally:
        bass.BassTensorEngine.preamble = pe_preamble

    x_d = nc.dram_tensor("x", [RPC, D], f32, kind="ExternalInput")
    lab_d = nc.dram_tensor("labels", [P, J], i32, kind="ExternalInput")
    cen_d = nc.dram_tensor("centers", [C, D], f32, kind="ExternalInput")
    out_d = nc.dram_tensor("out", [1, 1], f32, kind="ExternalOutput")

    with (
        ExitStack() as ctx,
        nc.sbuf_tensor("xt", [P, J, D], f32) as xt,
        nc.sbuf_tensor("ct", [P, J, D], f32) as ct,
        nc.sbuf_tensor("sq", [P, J, D], f32) as sq,
        nc.sbuf_tensor("sq2", [P, J, D], f32) as sq2,
        nc.sbuf_tensor("it", [P, J], i32) as it,
        nc.sbuf_tensor("dsum", [P, J], f32) as dsum,
        nc.sbuf_tensor("dclip", [P, J], f32) as dclip,
        nc.sbuf_tensor("dtot", [P, 1], f32) as dtot,
        nc.sbuf_tensor("res", [1, 1], f32) as res,
        nc.semaphore("s_idx") as s_idx,
        nc.semaphore("s_x") as s_x,
        nc.semaphore("s_v") as s_v,
        nc.semaphore("s_r") as s_r,
        nc.semaphore("s_out") as s_out,
    ):
        s_g = [ctx.enter_context(nc.semaphore(f"s_g{j}")) for j in range(J)]  # noqa: ANT232

        # ---- Sync: input DMAs up front, output DMA at the end
        nc.sync.dma_start(out=it[:], in_=lab_d[:, :]).then_inc(s_idx, 16)
        x_ap = x_d[:, :].rearrange("(p j) d -> p j d", p=P)
        nc.sync.dma_start(out=xt[:], in_=x_ap).then_inc(s_x, 16)
        nc.sync.wait_ge(s_r, 1)
        nc.sync.dma_start(out=out_d[:, :], in_=res[:]).then_inc(s_out, 16)
        nc.sync.wait_ge(s_out, 16)

        # ---- GpSimd: per-slot indirect gathers, then the cross-partition sum
        nc.gpsimd.wait_ge(s_idx, 16)
        for j in range(J):
            nc.gpsimd.indirect_dma_start(
                out=ct[:, j, :],
                out_offset=None,
                in_=cen_d[:, :],
                in_offset=bass.IndirectOffsetOnAxis(ap=it[:, j : j + 1], axis=0),
            ).then_inc(s_g[j], 16)
        nc.gpsimd.wait_ge(s_v, 1)
        nc.gpsimd.tensor_reduce(
            out=res[:],
            in_=dtot[:],
            axis=mybir.AxisListType.C,
            op=mybir.AluOpType.add,
        ).then_inc(s_r, 1)

        # ---- Vector: per-tile (x-c), then (x-c)^2/B with fused row-sum
        nc.vector.wait_ge(s_x, 16)
        for j in range(J):
            nc.vector.wait_ge(s_g[j], 16)
            nc.vector.tensor_tensor(
                out=sq[:, j, :],
                in0=xt[:, j, :],
                in1=ct[:, j, :],
                op=mybir.AluOpType.subtract,
            )
            nc.vector.drain()  # DVE pipeline: sq_j write -> read below
            nc.vector.scalar_tensor_tensor(
                out=sq2[:, j, :],
                in0=sq[:, j, :],
                scalar=1.0 / B,
                in1=sq[:, j, :],
                op0=mybir.AluOpType.mult,
                op1=mybir.AluOpType.mult,
                accum_out=dsum[:, j : j + 1],
            )
        nc.vector.drain()
        nc.vector.tensor_scalar(
            out=dclip[:],
            in0=dsum[:],
            scalar1=CLIP_LO / B,
            scalar2=CLIP_HI / B,
            op0=mybir.AluOpType.max,
            op1=mybir.AluOpType.min,
        )
        nc.vector.drain()
        nc.vector.tensor_reduce(
            out=dtot[:],
            in_=dclip[:],
            axis=mybir.AxisListType.X,
            op=mybir.AluOpType.add,
        )
        nc.vector.drain().then_inc(s_v, 1)

    nc.compile()
    return nc


def _get_nc():
    if "nc" not in _cache:
        _cache["nc"] = _build()
    return _cache["nc"]


def _make_in_maps(x, labels, centers):
    x = np.ascontiguousarray(np.asarray(x, dtype=np.float32))
    labels = np.asarray(labels).astype(np.int32)
    centers = np.ascontiguousarray(np.asarray(centers, dtype=np.float32))
    in_maps = []
    for i in range(N_CORES):
        sl = slice(i * RPC, (i + 1) * RPC)
        in_maps.append(
            {
                "x": x[sl],
                "labels": np.ascontiguousarray(labels[sl].reshape(P, J)),
                "centers": centers,
            }
        )
    return in_maps


def _run(in_maps, trace=False, **kwargs):
    from concourse.bass_utils import run_bass_kernel_spmd

    nc = _get_nc()
    return run_bass_kernel_spmd(
        nc, in_maps, core_ids=list(range(N_CORES)), trace=trace, **kwargs
    )


def kernel(x, labels, centers):
    res = _run(_make_in_maps(x, labels, centers))
    total = np.float32(0.0)
    for r in res.results:
        total += np.float32(r["out"].reshape(()))
    return np.asarray(total + np.float32(MASK_CONST), dtype=np.float32)
